# revision 21
# baseline (speedup 1.0000x reference)
"""BiAttention kernel for Trainium2, 8 NeuronCores, data-parallel over batch.

Reference computation (per batch b):
    S[i,j] = w1.c_i + w2.q_j + w3.(c_i*q_j)
    A      = softmax(S, axis=j)
    U[i]   = sum_j A[i,j] q_j
    bmax_i = max_j A[i,j]
    h      = sum_i bmax_i c_i
    G      = concat([c, U, c*U, c*H], axis=-1)

Two-phase schedule.  Phase 1 computes A^T = exp(S^T + s_q) for ALL row
blocks (cached in SBUF, bf16), plus row maxes, softmax denominators Z,
and h (accumulated incrementally, one block late in the PE stream so it
never waits on DVE results).  Phase 2 runs the U matmuls and assembles/
writes FULL 4KB G rows in a single DMA per row tile -- every quarter of
G, including c*h, is known by then, so the store stream is bandwidth-
bound (4KB descriptors) instead of 1KB-descriptor-rate-bound.

Key scheduling details:
  - ps_s has 4 PSUM buffers so the PE can run ~3us of non-S work
    (Z columns, c^T transposes, bmax transposes, h matmuls) between
    S bursts while the ACT engine drains queued exps.
  - phase 2's first row-tiles are emitted BEFORE the last block's
    bmax/h tail; only the c*h multiply and the row DMA depend on h, so
    the PE crosses the phase boundary without idling.
  - Z is accumulated directly in [128 i, tile] layout via ones-column
    matmuls (lhsT = A^T slices), avoiding a [1,512] PSUM row + flip.

Other restructurings:
  - softmax over j is invariant to w1.c_i -> w1 is dead.
  - S computed TRANSPOSED: S^T[j,i] = (w3*q)^T . c^T, so A^T lands in
    the layout the U matmul wants as lhsT (no A transposes).
  - +s_q[j] fused into exp as per-partition activation bias;
    s_q = qTw^T . (w2/w3) reuses the w3-scaled q^T.
  - no row-max subtraction (|S| <= ~12, fp32 PSUM exp is safe).
  - bmax over j (partition dim): 7-op DVE max tree + PE transpose +
    free-axis reduce.
  - all matmul operands bf16; accumulation fp32.
  - c -> bf16 via casting SBUF->SBUF DMAs on the gpsimd software DGE.
  - h accumulated as a [1,256] row (b_i stationary, c_bf moving), then
    partition-broadcast on GpSimd (no DRAM bounce).
"""

import sys

if "/opt/trn_rl_repo" not in sys.path:
    sys.path.insert(0, "/opt/trn_rl_repo")

from contextlib import ExitStack

import numpy as np

import concourse.bass as bass
import concourse.bacc as bacc_mod
import concourse.tile as tile
from concourse import mybir
from concourse.bass_utils import run_bass_kernel_spmd
from concourse.masks import make_identity

B, Tc, Tq, D = 8, 4096, 1024, 256
P = 128
NT = Tc // P  # 32 context row-tiles
JC = Tq // P  # 8 question partition-tiles
KC = D // P  # 2 feature chunks
IB = 4  # row-tiles per i-block
NB = NT // IB  # 8 i-blocks
IBW = IB * P  # 512 rows per block
N_CORES = 8
F32 = mybir.dt.float32
BF16 = mybir.dt.bfloat16
EXP = mybir.ActivationFunctionType.Exp
CPY = mybir.ActivationFunctionType.Copy
MAX = mybir.AluOpType.max
AXX = mybir.AxisListType.X


def _build_program() -> bass.Bass:
    nc = bacc_mod.Bacc()
    c_dram = nc.declare_dram_parameter("context", [Tc, D], F32, isOutput=False)
    q_dram = nc.declare_dram_parameter("question", [Tq, D], F32, isOutput=False)
    w_dram = nc.declare_dram_parameter("w", [3 * D, 1], F32, isOutput=False)
    g_dram = nc.declare_dram_parameter("out", [Tc, 4 * D], F32, isOutput=True)

    with ExitStack() as ctx:
        tc = ctx.enter_context(tile.TileContext(nc))
        singles = ctx.enter_context(tc.tile_pool(name="singles", bufs=1))
        work = ctx.enter_context(tc.tile_pool(name="work", bufs=3))
        g_pool = ctx.enter_context(tc.tile_pool(name="g", bufs=7))
        ps_s = ctx.enter_context(tc.tile_pool(name="ps_s", bufs=4, space="PSUM"))
        ps_tp = ctx.enter_context(tc.tile_pool(name="ps_tp", bufs=2, space="PSUM"))
        ps_zt = ctx.enter_context(tc.tile_pool(name="ps_zt", bufs=1, space="PSUM"))
        ps_h = ctx.enter_context(tc.tile_pool(name="ps_h", bufs=1, space="PSUM"))

        # ---------------- prep (once per batch element) ----------------
        c_all = singles.tile([P, NT, D], F32)
        c_bf = singles.tile([P, NT, D], BF16)
        AT_all = singles.tile([P, NB, JC, IBW], BF16)
        braw = singles.tile([P, NT], F32)
        b_bf = singles.tile([P, NT], BF16)
        rz_all = singles.tile([P, NT], F32)

        # earliest loads ride the ACT hardware queue (SP has a long init
        # preamble); later context blocks go to SP
        c_view = c_dram[:].rearrange("(p g) d -> p g d", g=NT)
        g_view = g_dram[:].rearrange("(p g) e -> p g e", g=NT)

        def load_c(b, eng=None):
            if b < NB:
                (eng or nc.sync).dma_start(
                    out=c_all[:, b * IB : (b + 1) * IB, :],
                    in_=c_view[:, b * IB : (b + 1) * IB, :],
                )

        ident = singles.tile([P, P], F32)
        make_identity(nc, ident)
        identb = singles.tile([P, P], BF16)
        nc.vector.tensor_copy(identb, ident)
        ones_col = singles.tile([P, 1], BF16)
        nc.vector.memset(ones_col, 1.0)
        one_one = singles.tile([1, 1], F32)
        nc.vector.memset(one_one, 1.0)

        q_raw = singles.tile([P, JC, D], F32)
        q_view = q_dram[:].rearrange("(p jj) d -> p jj d", jj=JC)
        nc.sync.dma_start(out=q_raw[:, 0:4, :], in_=q_view[:, 0:4, :])
        load_c(0)
        nc.sync.dma_start(out=q_raw[:, 4:8, :], in_=q_view[:, 4:8, :])
        load_c(1)

        w2c = singles.tile([P, KC], F32)
        w3c = singles.tile([P, KC], F32)
        for kc in range(KC):
            nc.scalar.dma_start(
                out=w2c[:, kc : kc + 1], in_=w_dram[D + kc * P : D + (kc + 1) * P, 0:1]
            )
            nc.scalar.dma_start(
                out=w3c[:, kc : kc + 1],
                in_=w_dram[2 * D + kc * P : 2 * D + (kc + 1) * P, 0:1],
            )


        w3r = singles.tile([P, KC], F32)
        nc.vector.reciprocal(w3r, w3c)
        w23 = singles.tile([P, KC], BF16)
        nc.vector.tensor_mul(w23, w2c, w3r)

        q_bf = singles.tile([P, JC, D], BF16)

        # q^T (w3-scaled, bf16) via PE transposes; s_q = qTw^T . (w2/w3)
        qTw = [singles.tile([P, Tq], BF16, name=f"qTw{k}") for k in range(KC)]
        sq_nat = singles.tile([P, JC], F32)
        for jg in range(2):
            sl = slice(jg * 512, (jg + 1) * 512)
            for kc in range(KC):
                tp = ps_tp.tile([P, 512], F32, tag="tp")
                for j4 in range(4):
                    jc = jg * 4 + j4
                    nc.tensor.transpose(
                        tp[:, j4 * P : (j4 + 1) * P],
                        q_raw[:, jc, kc * P : (kc + 1) * P],
                        ident,
                    )
                nc.scalar.activation(
                    qTw[kc][:, sl], tp, CPY, scale=w3c[:, kc : kc + 1]
                )
            sq_ps = ps_s.tile([P, 4], F32, tag="s")
            for j4 in range(4):
                jc = jg * 4 + j4
                for kc in range(KC):
                    nc.tensor.matmul(
                        sq_ps[:, j4 : j4 + 1],
                        lhsT=qTw[kc][:, jc * P : (jc + 1) * P],
                        rhs=w23[:, kc : kc + 1],
                        start=(kc == 0),
                        stop=(kc == KC - 1),
                    )
            nc.vector.tensor_copy(sq_nat[:, jg * 4 : (jg + 1) * 4], sq_ps)
        nc.vector.tensor_copy(q_bf, q_raw)

        # ---------------- phase 1: A^T, bmax, Z, h ----------------
        cT_tiles = {}
        m_tiles = {}
        z_tiles = {}
        h_ps = ps_h.tile([1, D], F32, tag="h")

        def to_bf(b):
            # c fp32 -> bf16 via casting SBUF->SBUF DMA (gpsimd swdge)
            if b < NB:
                t0 = b * IB
                nc.gpsimd.dma_start(
                    out=c_bf[:, t0 : t0 + IB, :], in_=c_all[:, t0 : t0 + IB, :]
                )

        def prep_ct(b):
            # PE transposes of c -> cT bf16 [128 d1, kc, 512 i]; DVE evac.
            # Blocks 0/1 transpose straight from fp32 c_all so the first S
            # matmuls don't wait on the bf16 casting DMA.
            if b >= NB:
                return
            t0 = b * IB
            src_c, idn = (c_all, ident) if b < 2 else (c_bf, identb)
            dt = F32 if b < 2 else BF16
            cT = work.tile([P, KC, IBW], BF16, tag="ct")
            for kc in range(KC):
                tp = ps_tp.tile([P, IBW], dt, tag="tp")
                for it in range(IB):
                    nc.tensor.transpose(
                        tp[:, it * P : (it + 1) * P],
                        src_c[:, t0 + it, kc * P : (kc + 1) * P],
                        idn,
                    )
                nc.vector.tensor_copy(cT[:, kc, :], tp)
            cT_tiles[b] = cT

        def bmax_tail(b):
            # one block late in the PE stream: bmax transpose + reduce,
            # 1/Z, b, and the h row matmuls for block b
            if not (0 <= b < NB):
                return
            t0 = b * IB
            m0 = m_tiles.pop(b)
            z_row = z_tiles.pop(b)
            zt4 = ps_tp.tile([P, IB], F32, tag="tp")
            for it in range(IB):
                nc.tensor.matmul(
                    zt4[:, it : it + 1],
                    lhsT=z_row[0:1, it * P : (it + 1) * P],
                    rhs=one_one,
                    start=True,
                    stop=True,
                )
            nc.vector.reciprocal(rz_all[:, t0 : t0 + IB], zt4)
            mx = ps_tp.tile([P, IB, P], BF16, tag="tp")
            for it in range(IB):
                nc.tensor.transpose(
                    mx[:, it, :], m0[:, it * P : (it + 1) * P], identb
                )
            nc.vector.tensor_reduce(
                out=braw[:, t0 : t0 + IB], in_=mx, axis=AXX, op=MAX
            )
            nc.vector.tensor_mul(
                b_bf[:, t0 : t0 + IB],
                braw[:, t0 : t0 + IB],
                rz_all[:, t0 : t0 + IB],
            )
            for it in range(IB):
                t = t0 + it
                nc.tensor.matmul(
                    h_ps,
                    lhsT=b_bf[:, t : t + 1],
                    rhs=c_bf[:, t, :],
                    start=(t == 0),
                    stop=(t == NT - 1),
                )

        to_bf(0)
        prep_ct(0)
        to_bf(1)

        for b in range(NB):
            t0 = b * IB
            load_c(b + 2)
            cT = cT_tiles.pop(b)

            # S^T[j, i] = (w3*q)^T . c^T ; A^T = exp(S^T + s_q[j]) per j-tile
            AT = AT_all[:, b, :, :]
            m0 = work.tile([P, IBW], BF16, tag="m0")
            m1 = work.tile([P, IBW], BF16, tag="m1")
            for jc in range(JC):
                s_ps = ps_s.tile([P, IBW], F32, tag="s")
                for kc in range(KC):
                    nc.tensor.matmul(
                        s_ps,
                        lhsT=qTw[kc][:, jc * P : (jc + 1) * P],
                        rhs=cT[:, kc, :],
                        start=(kc == 0),
                        stop=(kc == KC - 1),
                    )
                nc.scalar.activation(
                    AT[:, jc, :], s_ps, EXP, bias=sq_nat[:, jc : jc + 1]
                )
                if jc == 1:
                    nc.vector.tensor_max(m0, AT[:, 0, :], AT[:, 1, :])
                elif jc == 3:
                    nc.vector.tensor_max(m1, AT[:, 2, :], AT[:, 3, :])
                    nc.vector.tensor_max(m0, m0, m1)
                elif jc == 5:
                    nc.vector.tensor_max(m1, AT[:, 4, :], AT[:, 5, :])
                    nc.vector.tensor_max(m0, m0, m1)
                elif jc == 7:
                    nc.vector.tensor_max(m1, AT[:, 6, :], AT[:, 7, :])
                    nc.vector.tensor_max(m0, m0, m1)
            m_tiles[b] = m0

            # Z row = ones^T . A^T (one long-stream matmul per j-tile)
            z_ps = ps_zt.tile([1, IBW], F32, tag="zt")
            for jc in range(JC):
                nc.tensor.matmul(
                    z_ps,
                    lhsT=ones_col,
                    rhs=AT[:, jc, :],
                    start=(jc == 0),
                    stop=(jc == JC - 1),
                )
            z_row = work.tile([1, IBW], F32, tag="zrow")
            nc.vector.tensor_copy(z_row, z_ps)
            z_tiles[b] = z_row

            to_bf(b + 2)
            prep_ct(b + 1)

            # previous block's bmax/Z/h tail
            bmax_tail(b - 1)

        # ---------------- phase 2: U matmuls + full-row G writes ----------------
        # Only the c*h multiply and the row DMA depend on h, so the U
        # matmuls stream through the phase boundary while the last
        # block's bmax tail and the h broadcast complete.
        h_row = work.tile([1, D], F32, tag="hrow")
        h_bcast = singles.tile([P, D], F32)

        def u_compute(t):
            u_ps = ps_s.tile([P, D], F32, tag="s")
            AT = AT_all[:, t // IB, :, :]
            it = t % IB
            for jc in range(JC):
                nc.tensor.matmul(
                    u_ps,
                    lhsT=AT[:, jc, it * P : (it + 1) * P],
                    rhs=q_bf[:, jc, :],
                    start=(jc == 0),
                    stop=(jc == JC - 1),
                )
            g_sb = g_pool.tile([P, 4 * D], F32, tag="g")
            nc.scalar.activation(
                g_sb[:, D : 2 * D], u_ps, CPY, scale=rz_all[:, t : t + 1]
            )
            nc.vector.tensor_copy(g_sb[:, 0:D], c_all[:, t, :])
            eng = nc.gpsimd if t % 4 != 3 else nc.vector
            eng.tensor_mul(
                g_sb[:, 2 * D : 3 * D], c_all[:, t, :], g_sb[:, D : 2 * D]
            )
            return g_sb

        def g_finish(t, g_sb):
            nc.vector.tensor_mul(g_sb[:, 3 * D : 4 * D], c_all[:, t, :], h_bcast)
            nc.sync.dma_start(out=g_view[:, t, :], in_=g_sb)

        g0 = u_compute(0)
        g1 = u_compute(1)
        g2 = u_compute(2)
        g3 = u_compute(3)
        bmax_tail(NB - 1)
        g4 = u_compute(4)
        g5 = u_compute(5)
        nc.vector.tensor_copy(h_row, h_ps)
        nc.gpsimd.partition_broadcast(h_bcast, h_row[0:1, :], channels=P)
        for i, g in enumerate((g0, g1, g2, g3, g4, g5)):
            g_finish(i, g)
        for t in range(6, NT):
            g_finish(t, u_compute(t))

    nc.finalize()
    return nc


_NC_CACHE = None


def kernel(context, question, w):
    global _NC_CACHE
    context = np.asarray(context, dtype=np.float32)
    question = np.asarray(question, dtype=np.float32)
    w = np.asarray(w, dtype=np.float32)

    if _NC_CACHE is None:
        _NC_CACHE = _build_program()
    nc = _NC_CACHE

    in_maps = [
        {"context": context[b], "question": question[b], "w": w} for b in range(B)
    ]
    res = run_bass_kernel_spmd(nc, in_maps, list(range(N_CORES)))
    return np.stack([res.results[b]["out"] for b in range(B)], axis=0)


# revision 23
# speedup vs baseline: 1.2475x; 1.2475x over previous
"""BiAttention kernel for Trainium2, 8 NeuronCores, data-parallel over batch.

Reference computation (per batch b):
    S[i,j] = w1.c_i + w2.q_j + w3.(c_i*q_j)
    A      = softmax(S, axis=j)
    U[i]   = sum_j A[i,j] q_j
    bmax_i = max_j A[i,j]
    h      = sum_i bmax_i c_i
    G      = concat([c, U, c*U, c*H], axis=-1)

Two-phase schedule.  Phase 1 computes A^T = exp(S^T + s_q) for ALL row
blocks (cached in SBUF, bf16), plus row maxes, softmax denominators Z,
and h (accumulated incrementally, one block late in the PE stream so it
never waits on DVE results).  Phase 2 runs the U matmuls and assembles/
writes FULL 4KB G rows in a single DMA per row tile -- every quarter of
G, including c*h, is known by then, so the store stream is bandwidth-
bound (4KB descriptors) instead of 1KB-descriptor-rate-bound.

Key scheduling details:
  - ps_s has 4 PSUM buffers so the PE can run ~3us of non-S work
    (Z columns, c^T transposes, bmax transposes, h matmuls) between
    S bursts while the ACT engine drains queued exps.
  - phase 2's first row-tiles are emitted BEFORE the last block's
    bmax/h tail; only the c*h multiply and the row DMA depend on h, so
    the PE crosses the phase boundary without idling.
  - Z is accumulated directly in [128 i, tile] layout via ones-column
    matmuls (lhsT = A^T slices), avoiding a [1,512] PSUM row + flip.

Other restructurings:
  - softmax over j is invariant to w1.c_i -> w1 is dead.
  - S computed TRANSPOSED: S^T[j,i] = (w3*q)^T . c^T, so A^T lands in
    the layout the U matmul wants as lhsT (no A transposes).
  - +s_q[j] fused into exp as per-partition activation bias;
    s_q = qTw^T . (w2/w3) reuses the w3-scaled q^T.
  - no row-max subtraction (|S| <= ~12, fp32 PSUM exp is safe).
  - bmax over j (partition dim): 7-op DVE max tree + PE transpose +
    free-axis reduce.
  - all matmul operands bf16; accumulation fp32.
  - c -> bf16 via casting SBUF->SBUF DMAs on the gpsimd software DGE.
  - h accumulated as a [1,256] row (b_i stationary, c_bf moving), then
    partition-broadcast on GpSimd (no DRAM bounce).
"""

import sys

if "/opt/trn_rl_repo" not in sys.path:
    sys.path.insert(0, "/opt/trn_rl_repo")

from contextlib import ExitStack

import numpy as np

import concourse.bass as bass
import concourse.bacc as bacc_mod
import concourse.tile as tile
from concourse import mybir
from concourse.bass_utils import run_bass_kernel_spmd
from concourse.masks import make_identity

B, Tc, Tq, D = 8, 4096, 1024, 256
P = 128
NT = Tc // P  # 32 context row-tiles
JC = Tq // P  # 8 question partition-tiles
KC = D // P  # 2 feature chunks
IB = 4  # row-tiles per i-block
NB = NT // IB  # 8 i-blocks
IBW = IB * P  # 512 rows per block
N_CORES = 8
F32 = mybir.dt.float32
BF16 = mybir.dt.bfloat16
EXP = mybir.ActivationFunctionType.Exp
CPY = mybir.ActivationFunctionType.Copy
MAX = mybir.AluOpType.max
AXX = mybir.AxisListType.X


def _build_program() -> bass.Bass:
    nc = bacc_mod.Bacc()
    c_dram = nc.declare_dram_parameter("context", [Tc, D], F32, isOutput=False)
    q_dram = nc.declare_dram_parameter("question", [Tq, D], F32, isOutput=False)
    w_dram = nc.declare_dram_parameter("w", [3 * D, 1], F32, isOutput=False)
    g_dram = nc.declare_dram_parameter("out", [Tc, 4 * D], F32, isOutput=True)

    with ExitStack() as ctx:
        tc = ctx.enter_context(tile.TileContext(nc))
        singles = ctx.enter_context(tc.tile_pool(name="singles", bufs=1))
        work = ctx.enter_context(tc.tile_pool(name="work", bufs=3))
        g_pool = ctx.enter_context(tc.tile_pool(name="g", bufs=9))
        ps_s = ctx.enter_context(tc.tile_pool(name="ps_s", bufs=4, space="PSUM"))
        ps_tp = ctx.enter_context(tc.tile_pool(name="ps_tp", bufs=2, space="PSUM"))
        ps_zt = ctx.enter_context(tc.tile_pool(name="ps_zt", bufs=1, space="PSUM"))
        ps_h = ctx.enter_context(tc.tile_pool(name="ps_h", bufs=1, space="PSUM"))

        # ---------------- prep (once per batch element) ----------------
        c_all = singles.tile([P, NT, D], F32)
        c_bf = singles.tile([P, NT, D], BF16)
        AT_all = singles.tile([P, NB, JC, IBW], BF16)
        braw = singles.tile([P, NT], F32)
        b_bf = singles.tile([P, NT], BF16)
        rz_all = singles.tile([P, NT], F32)

        # earliest loads ride the ACT hardware queue (SP has a long init
        # preamble); later context blocks go to SP
        c_view = c_dram[:].rearrange("(p g) d -> p g d", g=NT)
        g_view = g_dram[:].rearrange("(p g) e -> p g e", g=NT)

        def load_c(b, eng=None):
            if b < NB:
                (eng or nc.sync).dma_start(
                    out=c_all[:, b * IB : (b + 1) * IB, :],
                    in_=c_view[:, b * IB : (b + 1) * IB, :],
                )

        ident = singles.tile([P, P], F32)
        make_identity(nc, ident)
        identb = singles.tile([P, P], BF16)
        nc.vector.tensor_copy(identb, ident)
        ones_col = singles.tile([P, 1], BF16)
        nc.vector.memset(ones_col, 1.0)

        q_raw = singles.tile([P, JC, D], F32)
        q_view = q_dram[:].rearrange("(p jj) d -> p jj d", jj=JC)
        nc.sync.dma_start(out=q_raw[:, 0:2, :], in_=q_view[:, 0:2, :])
        nc.sync.dma_start(out=q_raw[:, 2:4, :], in_=q_view[:, 2:4, :])
        load_c(0)
        nc.sync.dma_start(out=q_raw[:, 4:8, :], in_=q_view[:, 4:8, :])
        load_c(1)

        w2c = singles.tile([P, KC], F32)
        w3c = singles.tile([P, KC], F32)
        for kc in range(KC):
            nc.scalar.dma_start(
                out=w2c[:, kc : kc + 1], in_=w_dram[D + kc * P : D + (kc + 1) * P, 0:1]
            )
            nc.scalar.dma_start(
                out=w3c[:, kc : kc + 1],
                in_=w_dram[2 * D + kc * P : 2 * D + (kc + 1) * P, 0:1],
            )


        w3r = singles.tile([P, KC], F32)
        nc.vector.reciprocal(w3r, w3c)
        w23 = singles.tile([P, KC], BF16)
        nc.vector.tensor_mul(w23, w2c, w3r)

        q_bf = singles.tile([P, JC, D], BF16)

        # q^T (w3-scaled, bf16) via PE transposes; s_q = qTw^T . (w2/w3)
        qTw = [singles.tile([P, Tq], BF16, name=f"qTw{k}") for k in range(KC)]
        sq_nat = singles.tile([P, JC], F32)
        for jg in range(2):
            sl = slice(jg * 512, (jg + 1) * 512)
            for kc in range(KC):
                tp = ps_tp.tile([P, 512], F32, tag="tp")
                for j4 in range(4):
                    jc = jg * 4 + j4
                    nc.tensor.transpose(
                        tp[:, j4 * P : (j4 + 1) * P],
                        q_raw[:, jc : jc + 1, kc * P : (kc + 1) * P],
                        ident,
                    )
                nc.scalar.activation(
                    qTw[kc][:, sl], tp, CPY, scale=w3c[:, kc : kc + 1]
                )
            sq_ps = ps_s.tile([P, 4], F32, tag="s")
            for j4 in range(4):
                jc = jg * 4 + j4
                for kc in range(KC):
                    nc.tensor.matmul(
                        sq_ps[:, j4 : j4 + 1],
                        lhsT=qTw[kc][:, jc * P : (jc + 1) * P],
                        rhs=w23[:, kc : kc + 1],
                        start=(kc == 0),
                        stop=(kc == KC - 1),
                    )
            nc.vector.tensor_copy(sq_nat[:, jg * 4 : (jg + 1) * 4], sq_ps)
        nc.vector.tensor_copy(q_bf, q_raw)

        # ---------------- phase 1: A^T, bmax, Z, h ----------------
        cT_tiles = {}
        m_tiles = {}
        h_ps = ps_h.tile([1, D], F32, tag="h")

        def to_bf(b):
            # c fp32 -> bf16 via casting SBUF->SBUF DMA (gpsimd swdge)
            if b < NB:
                t0 = b * IB
                nc.gpsimd.dma_start(
                    out=c_bf[:, t0 : t0 + IB, :], in_=c_all[:, t0 : t0 + IB, :]
                )

        def prep_ct(b):
            # PE transposes of c -> cT bf16 [128 d1, kc, 512 i]; DVE evac.
            # Blocks 0/1 transpose straight from fp32 c_all so the first S
            # matmuls don't wait on the bf16 casting DMA.
            if b >= NB:
                return
            t0 = b * IB
            src_c, idn = (c_all, ident) if b < 2 else (c_bf, identb)
            dt = F32 if b < 2 else BF16
            cT = work.tile([P, KC, IBW], BF16, tag="ct")
            for kc in range(KC):
                tp = ps_tp.tile([P, IBW], dt, tag="tp")
                for it in range(IB):
                    nc.tensor.transpose(
                        tp[:, it * P : (it + 1) * P],
                        src_c[:, t0 + it, kc * P : (kc + 1) * P],
                        idn,
                    )
                nc.vector.tensor_copy(cT[:, kc, :], tp)
            cT_tiles[b] = cT

        def bmax_tail(b):
            # one block late in the PE stream: bmax transpose + reduce,
            # 1/Z, b, and the h row matmuls for block b
            if not (0 <= b < NB):
                return
            t0 = b * IB
            m0 = m_tiles.pop(b)
            mx = ps_tp.tile([P, IB, P], BF16, tag="tp")
            for it in range(IB):
                nc.tensor.transpose(
                    mx[:, it, :], m0[:, it * P : (it + 1) * P], identb
                )
            nc.vector.tensor_reduce(
                out=braw[:, t0 : t0 + IB], in_=mx, axis=AXX, op=MAX
            )
            nc.vector.tensor_mul(
                b_bf[:, t0 : t0 + IB],
                braw[:, t0 : t0 + IB],
                rz_all[:, t0 : t0 + IB],
            )
            for it in range(IB):
                t = t0 + it
                nc.tensor.matmul(
                    h_ps,
                    lhsT=b_bf[:, t : t + 1],
                    rhs=c_bf[:, t, :],
                    start=(t == 0),
                    stop=(t == NT - 1),
                )

        to_bf(0)
        prep_ct(0)
        to_bf(1)

        for b in range(NB):
            t0 = b * IB
            load_c(b + 2)
            cT = cT_tiles.pop(b)

            # S^T[j, i] = (w3*q)^T . c^T ; A^T = exp(S^T + s_q[j]) per j-tile
            AT = AT_all[:, b, :, :]
            m0 = work.tile([P, IBW], BF16, tag="m0")
            m1 = work.tile([P, IBW], BF16, tag="m1")
            for jc in range(JC):
                s_ps = ps_s.tile([P, IBW], F32, tag="s")
                for kc in range(KC):
                    nc.tensor.matmul(
                        s_ps,
                        lhsT=qTw[kc][:, jc * P : (jc + 1) * P],
                        rhs=cT[:, kc, :],
                        start=(kc == 0),
                        stop=(kc == KC - 1),
                    )
                nc.scalar.activation(
                    AT[:, jc, :], s_ps, EXP, bias=sq_nat[:, jc : jc + 1]
                )
                if jc == 1:
                    nc.vector.tensor_max(m0, AT[:, 0, :], AT[:, 1, :])
                elif jc == 3:
                    nc.vector.tensor_max(m1, AT[:, 2, :], AT[:, 3, :])
                    nc.vector.tensor_max(m0, m0, m1)
                elif jc == 5:
                    nc.vector.tensor_max(m1, AT[:, 4, :], AT[:, 5, :])
                    nc.vector.tensor_max(m0, m0, m1)
                elif jc == 7:
                    nc.vector.tensor_max(m1, AT[:, 6, :], AT[:, 7, :])
                    nc.vector.tensor_max(m0, m0, m1)
            m_tiles[b] = m0

            # Z columns: zt[:, it] = sum_j A^T[j, i-tile]  (A^T stationary)
            zt = ps_zt.tile([P, IB], F32, tag="zt")
            for it in range(IB):
                for jc in range(JC):
                    nc.tensor.matmul(
                        zt[:, it : it + 1],
                        lhsT=AT[:, jc, it * P : (it + 1) * P],
                        rhs=ones_col,
                        start=(jc == 0),
                        stop=(jc == JC - 1),
                    )
            nc.vector.reciprocal(rz_all[:, t0 : t0 + IB], zt)

            to_bf(b + 2)
            prep_ct(b + 1)

            # previous block's bmax/Z/h tail
            bmax_tail(b - 1)

        # ---------------- phase 2: U matmuls + full-row G writes ----------------
        # Only the c*h multiply and the row DMA depend on h, so the U
        # matmuls stream through the phase boundary while the last
        # block's bmax tail and the h broadcast complete.
        h_row = work.tile([1, D], F32, tag="hrow")
        h_bcast = singles.tile([P, D], F32)

        def u_compute(t):
            u_ps = ps_s.tile([P, D], F32, tag="s")
            AT = AT_all[:, t // IB, :, :]
            it = t % IB
            for jc in range(JC):
                nc.tensor.matmul(
                    u_ps,
                    lhsT=AT[:, jc, it * P : (it + 1) * P],
                    rhs=q_bf[:, jc, :],
                    start=(jc == 0),
                    stop=(jc == JC - 1),
                )
            g_sb = g_pool.tile([P, 4 * D], F32, tag="g")
            nc.scalar.activation(
                g_sb[:, D : 2 * D], u_ps, CPY, scale=rz_all[:, t : t + 1]
            )
            nc.vector.tensor_copy(g_sb[:, 0:D], c_all[:, t, :])
            eng = nc.gpsimd if t % 4 != 3 else nc.vector
            eng.tensor_mul(
                g_sb[:, 2 * D : 3 * D], c_all[:, t, :], g_sb[:, D : 2 * D]
            )
            return g_sb

        def g_finish(t, g_sb):
            nc.vector.tensor_mul(g_sb[:, 3 * D : 4 * D], c_all[:, t, :], h_bcast)
            nc.sync.dma_start(out=g_view[:, t, :], in_=g_sb)

        g0 = u_compute(0)
        g1 = u_compute(1)
        bmax_tail(NB - 1)
        nc.vector.tensor_copy(h_row, h_ps)
        nc.gpsimd.partition_broadcast(h_bcast, h_row[0:1, :], channels=P)
        g2 = u_compute(2)
        g_finish(0, g0)
        g3 = u_compute(3)
        g_finish(1, g1)
        g_finish(2, g2)
        g_finish(3, g3)
        for t in range(4, NT):
            g_finish(t, u_compute(t))

    nc.finalize()
    return nc


_NC_CACHE = None


def kernel(context, question, w):
    global _NC_CACHE
    context = np.asarray(context, dtype=np.float32)
    question = np.asarray(question, dtype=np.float32)
    w = np.asarray(w, dtype=np.float32)

    if _NC_CACHE is None:
        _NC_CACHE = _build_program()
    nc = _NC_CACHE

    in_maps = [
        {"context": context[b], "question": question[b], "w": w} for b in range(B)
    ]
    res = run_bass_kernel_spmd(nc, in_maps, list(range(N_CORES)))
    return np.stack([res.results[b]["out"] for b in range(B)], axis=0)
